# revision 12
# baseline (speedup 1.0000x reference)
"""Trainium2 Bass kernel for a pre-norm transformer block (B=1, T=4096, C=1024, H=16).

Sharding (8 cores): head-tensor-parallel attention (2 heads/core) with
data-parallel (sequence-local) MLP. The host broadcasts the FULL input
x (bf16, transposed [C, T]) to every core; each core computes the
LayerNorm over all T locally (redundantly), so the expensive h1
AllGather of the previous version is eliminated. The only collective is
a small bf16 AllToAll of the per-head attention outputs.

Everything is bf16 (weights host-cast) except PSUM accumulation and LN
statistics. Attention scores use PE row-tiling: head0 occupies
partitions 0:64 and head1 64:128 of q/k, so one 512-col moving pass
computes both heads' score blocks concurrently. The two heads' PV
accumulators share one PSUM bank ([128, 130] = 2 x (64 o + 1 l)) using
per-element has_written semantics (single start=True clears the bank).
"""
import numpy as np
import ml_dtypes

import concourse.bass as bass
import concourse.bacc as bacc
import concourse.tile as tile
import concourse.mybir as mybir
from concourse import bass_utils

F32 = mybir.dt.float32
BF16 = mybir.dt.bfloat16
AF = mybir.ActivationFunctionType
OP = mybir.AluOpType

NCORES = 8
C = 1024
T = 4096
TC = T // NCORES          # 512 own tokens
CK = C // 128             # 8 C chunks
FC = 4096                 # MLP hidden
EPS = 1e-5

_CACHE = {}


def _layer_norm_own(nc, tc, sb, x_t, w_ap, out_t, eps_t, ones_t, tag):
    """LN over C for the own-column tile x_t [128, CK*512] bf16 (chunk k at
    cols 512k). out_t bf16 same layout. Stats in fp32."""
    with tc.tile_pool(name=f"ln{tag}", bufs=1) as lnp, \
         tc.tile_pool(name=f"lnps{tag}", bufs=1, space="PSUM") as lps:
        mean_ps = lps.tile([128, 512], F32, name=f"mean{tag}")
        sq_ps = lps.tile([128, 512], F32, name=f"sqs{tag}")
        for k in range(CK):
            xs = x_t[:, 512 * k:512 * (k + 1)]
            nc.tensor.matmul(mean_ps[:], ones_t[:], xs,
                             start=(k == 0), stop=(k == CK - 1))
            sq = lnp.tile([128, 512], BF16, name=f"sq{tag}", bufs=4)
            nc.vector.tensor_mul(sq[:], xs, xs)
            nc.tensor.matmul(sq_ps[:], ones_t[:], sq[:],
                             start=(k == 0), stop=(k == CK - 1))
        mu = lnp.tile([128, 512], F32, name=f"mu{tag}")
        nc.vector.tensor_scalar_mul(mu[:], mean_ps[:], 1.0 / C)
        musq = lnp.tile([128, 512], F32, name=f"musq{tag}")
        nc.vector.tensor_mul(musq[:], mu[:], mu[:])
        var = lnp.tile([128, 512], F32, name=f"var{tag}")
        nc.vector.scalar_tensor_tensor(var[:], sq_ps[:], 1.0 / C, musq[:],
                                       OP.mult, OP.subtract)
        lnv = lnp.tile([128, 512], F32, name=f"lnv{tag}")
        nc.scalar.activation(lnv[:], var[:], AF.Ln, bias=eps_t[:])
        rstd = lnp.tile([128, 512], F32, name=f"rstd{tag}")
        nc.scalar.activation(rstd[:], lnv[:], AF.Exp, scale=-0.5)
        for k in range(CK):
            d = lnp.tile([128, 512], F32, name=f"d{tag}", bufs=4)
            nc.vector.tensor_sub(d[:], x_t[:, 512 * k:512 * (k + 1)], mu[:])
            nc.vector.scalar_tensor_tensor(
                out_t[:, 512 * k:512 * (k + 1)], d[:], w_ap[:, k:k + 1],
                rstd[:], OP.mult, OP.mult)


def _build():
    nc = bacc.Bacc("TRN2", target_bir_lowering=False, debug=False,
                   enable_asserts=False, num_devices=NCORES)

    xT = nc.dram_tensor("xT", [C, T], BF16, kind="ExternalInput").ap()
    xoT = nc.dram_tensor("xoT", [C, TC], BF16, kind="ExternalInput").ap()
    wqkv = nc.dram_tensor("wqkv", [C, 384], BF16, kind="ExternalInput").ap()
    wproj = nc.dram_tensor("wproj", [C, C], BF16, kind="ExternalInput").ap()
    wfc = nc.dram_tensor("wfc", [C, FC], BF16, kind="ExternalInput").ap()
    wmlp = nc.dram_tensor("wmlp", [FC, C], BF16, kind="ExternalInput").ap()
    ln1w = nc.dram_tensor("ln1w", [128, CK], F32, kind="ExternalInput").ap()
    ln2w = nc.dram_tensor("ln2w", [128, CK], F32, kind="ExternalInput").ap()
    masks = nc.dram_tensor("masks", [128, 4 * 1024], BF16,
                           kind="ExternalInput").ap()
    ident = nc.dram_tensor("ident", [128, 128], BF16, kind="ExternalInput").ap()
    onesw = nc.dram_tensor("onesw", [128, 128], BF16, kind="ExternalInput").ap()
    outT = nc.dram_tensor("outT", [C, TC], BF16, kind="ExternalOutput").ap()

    rg = [list(range(NCORES))]

    with tile.TileContext(nc) as tc:
        with tc.tile_pool(name="dram", bufs=1, space="DRAM") as dramp:
            ob = dramp.tile([C, TC], BF16, name="ob")
            oax = dramp.tile([C, TC], BF16, name="oax")

            with tc.tile_pool(name="glob", bufs=1) as gp:
                ident_t = gp.tile([128, 128], BF16, name="ident_t")
                nc.sync.dma_start(ident_t[:], ident[:])
                ones_t = gp.tile([128, 128], BF16, name="ones_t")
                nc.sync.dma_start(ones_t[:], onesw[:])
                ln1w_t = gp.tile([128, CK], F32, name="ln1w_t")
                nc.sync.dma_start(ln1w_t[:], ln1w[:])
                ln2w_t = gp.tile([128, CK], F32, name="ln2w_t")
                nc.sync.dma_start(ln2w_t[:], ln2w[:])
                eps_t = gp.tile([128, 1], F32, name="eps_t")
                nc.vector.memset(eps_t[:], EPS)
                masks_t = gp.tile([128, 4 * 1024], BF16, name="masks_t")
                x_own = gp.tile([128, CK * TC], BF16, name="x_own")
                oT = gp.tile([128, T], BF16, name="oT")

                # ---------------- attention scope ----------------
                with tc.tile_pool(name="attn", bufs=1) as ap:
                    qp = ap.tile([128, T], BF16, name="qp")
                    kT = ap.tile([128, T], BF16, name="kT")
                    vT = ap.tile([128, T], BF16, name="vT")

                    # LN1 over FULL T + QKV for the 2 local heads.
                    # LN is applied IN-PLACE into x_t (SBUF budget): after
                    # the apply loop, x_t holds h1.
                    with tc.tile_pool(name="p1", bufs=1) as p1:
                        x_t = p1.tile([128, CK * T], BF16, name="x_t")
                        mu = p1.tile([128, T], BF16, name="mu")
                        rstd = p1.tile([128, T], BF16, name="rstd")
                        # x chunks first (LN1 stats are the critical path),
                        # split in halves for earlier first-arrival
                        for k in range(CK):
                            for hh in range(2):
                                nc.sync.dma_start(
                                    x_t[:, T * k + 2048 * hh:
                                        T * k + 2048 * (hh + 1)],
                                    xT[128 * k:128 * (k + 1),
                                       2048 * hh:2048 * (hh + 1)])
                        wq_t = []
                        for k in range(CK):
                            w = p1.tile([128, 384], BF16, name=f"wq{k}")
                            nc.sync.dma_start(w[:],
                                              wqkv[128 * k:128 * (k + 1), :])
                            wq_t.append(w)
                        nc.sync.dma_start(masks_t[:], masks[:])
                        for k in range(CK):
                            nc.sync.dma_start(x_own[:, 512 * k:512 * (k + 1)],
                                              xoT[128 * k:128 * (k + 1), :])
                        # stats in 2 halves of T (PSUM: 4+4 banks per half)
                        with tc.tile_pool(name="lnps1", bufs=1,
                                          space="PSUM") as lps:
                            for half in range(2):
                                mps = [lps.tile([128, 512], F32, name=f"m{tb}",
                                                bufs=1) for tb in range(4)]
                                sps = [lps.tile([128, 512], F32, name=f"s{tb}",
                                                bufs=1) for tb in range(4)]
                                for k in range(CK):
                                    for tb in range(4):
                                        col = T * k + 2048 * half + 512 * tb
                                        xs = x_t[:, col:col + 512]
                                        nc.tensor.matmul(
                                            mps[tb][:], ones_t[:], xs,
                                            start=(k == 0), stop=(k == CK - 1))
                                        sq = p1.tile([128, 512], BF16,
                                                     name="sq1", bufs=4)
                                        nc.vector.tensor_mul(sq[:], xs, xs)
                                        nc.tensor.matmul(
                                            sps[tb][:], ones_t[:], sq[:],
                                            start=(k == 0), stop=(k == CK - 1))
                                for tb in range(4):
                                    tcol = 2048 * half + 512 * tb
                                    mus = mu[:, tcol:tcol + 512]
                                    nc.vector.tensor_scalar_mul(
                                        mus, mps[tb][:], 1.0 / C)
                                    musq = p1.tile([128, 512], F32,
                                                   name="musq1", bufs=2)
                                    nc.vector.tensor_mul(musq[:], mus, mus)
                                    var = p1.tile([128, 512], F32,
                                                  name="var1", bufs=2)
                                    nc.vector.scalar_tensor_tensor(
                                        var[:], sps[tb][:], 1.0 / C, musq[:],
                                        OP.mult, OP.subtract)
                                    lnv = p1.tile([128, 512], F32,
                                                  name="lnv1", bufs=2)
                                    nc.scalar.activation(lnv[:], var[:],
                                                         AF.Ln, bias=eps_t[:])
                                    nc.scalar.activation(
                                        rstd[:, tcol:tcol + 512], lnv[:],
                                        AF.Exp, scale=-0.5)
                        # apply LN1 to all chunks, in place (x_t becomes h1)
                        for k in range(CK):
                            for tb in range(8):
                                col = T * k + 512 * tb
                                d = p1.tile([128, 512], BF16, name="d1",
                                            bufs=4)
                                nc.vector.tensor_sub(
                                    d[:], x_t[:, col:col + 512],
                                    mu[:, 512 * tb:512 * (tb + 1)])
                                nc.vector.scalar_tensor_tensor(
                                    x_t[:, col:col + 512], d[:],
                                    ln1w_t[:, k:k + 1],
                                    rstd[:, 512 * tb:512 * (tb + 1)],
                                    OP.mult, OP.mult)
                        # qkv: dst m=0 -> qp, 1 -> kT, 2 -> vT. j-outer with
                        # double-buffered PSUM so the psum->SBUF copies of
                        # block j overlap the matmuls of block j+1.
                        dsts = [qp, kT, vT]
                        with tc.tile_pool(name="qkps", bufs=1,
                                          space="PSUM") as qps:
                            for j in range(8):
                                for m in range(3):
                                    pm = qps.tile([128, 512], F32,
                                                  name=f"pm{m}", bufs=2)
                                    for k in range(CK):
                                        nc.tensor.matmul(
                                            pm[:],
                                            wq_t[k][:, 128 * m:128 * (m + 1)],
                                            x_t[:, T * k + 512 * j:
                                                T * k + 512 * (j + 1)],
                                            start=(k == 0), stop=(k == CK - 1))
                                    nc.vector.tensor_copy(
                                        dsts[m][:, 512 * j:512 * (j + 1)],
                                        pm[:])

                    # v transposed per head: ve[h] [128, 32*65] bf16
                    # (block kb at cols 65*kb; col 64 of each block = ones)
                    ve = [ap.tile([128, 32 * 65], BF16, name=f"ve{h}")
                          for h in range(2)]
                    for h in range(2):
                        nc.vector.memset(ve[h][:], 1.0)
                    with tc.tile_pool(name="veps", bufs=1, space="PSUM") as vps:
                        for t in range(T // 128):
                            tp = vps.tile([128, 128], BF16, name="vtp", bufs=4)
                            nc.tensor.transpose(tp[:],
                                                vT[:, 128 * t:128 * (t + 1)],
                                                ident_t[:])
                            for h in range(2):
                                nc.vector.tensor_copy(
                                    ve[h][:, 65 * t:65 * t + 64],
                                    tp[:, 64 * h:64 * (h + 1)])

                    # flash attention (no max subtraction), both heads/pass
                    o2_tiles = []
                    with tc.tile_pool(name="atw", bufs=1) as aw:
                      with tc.tile_pool(name="atps", bufs=1,
                                        space="PSUM") as aps:
                        for qi in range(8):
                            nkb = 4 * (qi + 1)
                            ops = [aps.tile([128, 130], F32, name=f"ops{q}",
                                            bufs=1) for q in range(4)]
                            for kb in range(nkb):
                                st = aps.tile([128, 1024], F32, name="st",
                                              bufs=2)
                                for h in range(2):
                                    nc.tensor.matmul(
                                        st[:, 512 * h:512 * (h + 1)],
                                        kT[64 * h:64 * (h + 1),
                                           128 * kb:128 * (kb + 1)],
                                        qp[64 * h:64 * (h + 1),
                                           512 * qi:512 * (qi + 1)],
                                        start=True, stop=True)
                                est = aw.tile([128, 1024], BF16, name="est",
                                              bufs=4)
                                nc.scalar.activation(est[:], st[:], AF.Exp,
                                                     scale=0.125)
                                if kb >= 4 * qi:
                                    jm = kb - 4 * qi
                                    nc.vector.tensor_mul(
                                        est[:], est[:],
                                        masks_t[:, 1024 * jm:1024 * (jm + 1)])
                                last = (kb == nkb - 1)
                                for q in range(4):
                                    nc.tensor.matmul(
                                        ops[q][:, 0:65],
                                        est[:, 128 * q:128 * (q + 1)],
                                        ve[0][:, 65 * kb:65 * kb + 65],
                                        start=(kb == 0), stop=last,
                                        skip_group_check=True)
                                    nc.tensor.matmul(
                                        ops[q][:, 65:130],
                                        est[:, 512 + 128 * q:512 + 128 * (q + 1)],
                                        ve[1][:, 65 * kb:65 * kb + 65],
                                        start=False, stop=last,
                                        skip_group_check=True)
                            # normalize: o2[q] = [o_h0 * 1/l_h0 | o_h1 * 1/l_h1]
                            l_sb = aw.tile([128, 8], F32, name="l_sb", bufs=2)
                            for q in range(4):
                                for h in range(2):
                                    nc.vector.tensor_copy(
                                        l_sb[:, 2 * q + h:2 * q + h + 1],
                                        ops[q][:, 65 * h + 64:65 * h + 65])
                            rl = aw.tile([128, 8], F32, name="rl", bufs=2)
                            nc.vector.reciprocal(rl[:], l_sb[:])
                            for q in range(4):
                                o2 = aw.tile([128, 128], BF16,
                                             name=f"o2_{qi}_{q}")
                                for h in range(2):
                                    nc.vector.tensor_scalar_mul(
                                        o2[:, 64 * h:64 * (h + 1)],
                                        ops[q][:, 65 * h:65 * h + 64],
                                        rl[:, 2 * q + h:2 * q + h + 1])
                                o2_tiles.append(o2)
                      # transpose all o2 -> oT [d2, q] (atps closed: banks free)
                      with tc.tile_pool(name="trps", bufs=1,
                                        space="PSUM") as tq:
                          for i, o2 in enumerate(o2_tiles):
                              tp2 = tq.tile([128, 128], BF16, name="tp2",
                                            bufs=4)
                              nc.tensor.transpose(tp2[:], o2[:], ident_t[:])
                              nc.vector.tensor_copy(
                                  oT[:, 128 * i:128 * (i + 1)], tp2[:])

                # tail scope: proj + MLP. Weight prefetch DMAs are issued
                # BEFORE the collective so they overlap the a2a latency.
                with tc.tile_pool(name="tail", bufs=1) as tp:
                    x2_t = tp.tile([128, CK * TC], BF16, name="x2_t")
                    out_sb = tp.tile([128, CK * TC], BF16, name="out_sb")
                    wp_t = []
                    for k in range(CK):
                        w = tp.tile([128, C], BF16, name=f"wp{k}")
                        nc.sync.dma_start(w[:],
                                          wproj[128 * k:128 * (k + 1), :])
                        wp_t.append(w)
                    wfc_r = wfc.rearrange("(k p) h -> p k h", p=128)
                    wg_t = []
                    for m in range(FC // 128):
                        w = tp.tile([128, CK, 128], BF16, name=f"wg{m}")
                        nc.sync.dma_start(
                            w[:], wfc_r[:, :, 128 * m:128 * (m + 1)])
                        wg_t.append(w)

                    # exchange head outputs: AllToAll (bf16)
                    for j in range(NCORES):
                        nc.sync.dma_start(ob[128 * j:128 * (j + 1), :],
                                          oT[:, 512 * j:512 * (j + 1)])
                    nc.gpsimd.collective_compute(
                        "AllToAll", OP.bypass, replica_groups=rg,
                        ins=[ob.opt()], outs=[oax.opt()])

                    # proj: x2 = x_own + wproj.T @ aout (own columns)
                    with tc.tile_pool(name="prs", bufs=1) as prs, \
                         tc.tile_pool(name="prps", bufs=1, space="PSUM") as pps:
                        x2ps = [pps.tile([128, 512], F32, name=f"x2p{m}")
                                for m in range(CK)]
                        for k in range(CK):
                            at = prs.tile([128, 512], BF16, name="at", bufs=3)
                            nc.sync.dma_start(at[:],
                                              oax[128 * k:128 * (k + 1), :])
                            for m in range(CK):
                                nc.tensor.matmul(
                                    x2ps[m][:],
                                    wp_t[k][:, 128 * m:128 * (m + 1)],
                                    at[:], start=(k == 0), stop=(k == CK - 1))
                        for m in range(CK):
                            nc.vector.tensor_add(
                                x2_t[:, 512 * m:512 * (m + 1)], x2ps[m][:],
                                x_own[:, 512 * m:512 * (m + 1)])

                    # ---------------- MLP ----------------
                    with tc.tile_pool(name="mlp", bufs=1) as mp:
                        h2 = mp.tile([128, CK * TC], BF16, name="h2")
                        _layer_norm_own(nc, tc, mp, x2_t, ln2w_t, h2, eps_t,
                                        ones_t, "2")
                        gel = []
                        with tc.tile_pool(name="fcps", bufs=1,
                                          space="PSUM") as fps:
                            for m in range(FC // 128):
                                pf = fps.tile([128, 512], F32, name="fcp",
                                              bufs=2)
                                for k in range(CK):
                                    nc.tensor.matmul(
                                        pf[:], wg_t[m][:, k, :],
                                        h2[:, 512 * k:512 * (k + 1)],
                                        start=(k == 0), stop=(k == CK - 1))
                                g = mp.tile([128, 512], BF16, name=f"gel{m}")
                                nc.scalar.activation(g[:], pf[:], AF.Gelu)
                                gel.append(g)
                        # second matmul in two half-passes (PSUM budget)
                        with tc.tile_pool(name="m2s", bufs=1) as m2s, \
                             tc.tile_pool(name="m2ps", bufs=1,
                                          space="PSUM") as m2ps:
                            for half in range(2):
                                x3ps = [m2ps.tile([128, 512], F32,
                                                  name=f"x3p{i}", bufs=1)
                                        for i in range(4)]
                                for h in range(FC // 128):
                                    wm = m2s.tile([128, 512], BF16, name="wm",
                                                  bufs=8)
                                    nc.sync.dma_start(
                                        wm[:],
                                        wmlp[128 * h:128 * (h + 1),
                                             512 * half:512 * (half + 1)])
                                    for i in range(4):
                                        nc.tensor.matmul(
                                            x3ps[i][:],
                                            wm[:, 128 * i:128 * (i + 1)],
                                            gel[h][:], start=(h == 0),
                                            stop=(h == FC // 128 - 1))
                                for i in range(4):
                                    m = 4 * half + i
                                    nc.vector.tensor_add(
                                        out_sb[:, 512 * m:512 * (m + 1)],
                                        x3ps[i][:],
                                        x2_t[:, 512 * m:512 * (m + 1)])
                                    nc.sync.dma_start(
                                        outT[128 * m:128 * (m + 1), :],
                                        out_sb[:, 512 * m:512 * (m + 1)])

    nc.compile()
    return nc


def _host_inputs(x, w_qkv, w_attn_proj, w_fc, w_mlp_proj, ln1_w, ln2_w):
    """Build the 8 per-core input maps (bf16 weights/activations)."""
    bf16 = ml_dtypes.bfloat16
    x2 = np.ascontiguousarray(np.asarray(x, np.float32).reshape(T, C))
    xT_full = np.ascontiguousarray(x2.T).astype(bf16)
    w_qkv = np.asarray(w_qkv, np.float32)
    masks = np.zeros((128, 4 * 1024), np.float32)
    kk = np.arange(128)[:, None]
    qq = np.arange(512)[None, :]
    for jm in range(4):
        m = (qq >= kk + 128 * jm)
        masks[:, 1024 * jm:1024 * jm + 512] = m
        masks[:, 1024 * jm + 512:1024 * (jm + 1)] = m
    masks = masks.astype(bf16)
    ident = np.eye(128, dtype=np.float32).astype(bf16)
    onesw = np.ones((128, 128), np.float32).astype(bf16)
    ln1 = np.ascontiguousarray(np.asarray(ln1_w, np.float32).reshape(CK, 128).T)
    ln2 = np.ascontiguousarray(np.asarray(ln2_w, np.float32).reshape(CK, 128).T)
    common = {
        "xT": xT_full,
        "wproj": np.ascontiguousarray(
            np.asarray(w_attn_proj, np.float32)).astype(bf16),
        "wfc": np.ascontiguousarray(np.asarray(w_fc, np.float32)).astype(bf16),
        "wmlp": np.ascontiguousarray(
            np.asarray(w_mlp_proj, np.float32)).astype(bf16),
        "ln1w": ln1, "ln2w": ln2, "masks": masks, "ident": ident,
        "onesw": onesw,
    }
    in_maps = []
    for c in range(NCORES):
        xoT = np.ascontiguousarray(x2[TC * c:TC * (c + 1), :].T).astype(bf16)
        wq = np.ascontiguousarray(np.concatenate(
            [w_qkv[:, C * s + 128 * c:C * s + 128 * (c + 1)] for s in range(3)],
            axis=1)).astype(bf16)
        in_maps.append({"xoT": xoT, "wqkv": wq, **common})
    return in_maps


def _run(in_maps, **kw):
    if "nc" not in _CACHE:
        _CACHE["nc"] = _build()
    return bass_utils.run_bass_kernel_spmd(
        _CACHE["nc"], in_maps, core_ids=list(range(NCORES)), **kw)


def kernel(x, w_qkv, w_attn_proj, w_fc, w_mlp_proj, ln1_w, ln2_w):
    in_maps = _host_inputs(x, w_qkv, w_attn_proj, w_fc, w_mlp_proj,
                           ln1_w, ln2_w)
    res = _run(in_maps)
    out = np.empty((1, T, C), np.float32)
    for c in range(NCORES):
        out[0, TC * c:TC * (c + 1), :] = \
            res.results[c]["outT"].astype(np.float32).T
    return out


# revision 18
# speedup vs baseline: 1.1664x; 1.1664x over previous
"""Trainium2 Bass kernel for a pre-norm transformer block (B=1, T=4096, C=1024, H=16).

Sharding (8 cores): head-tensor-parallel attention (2 heads/core) with
data-parallel (sequence-local) MLP. The host broadcasts the FULL input
x (bf16, transposed [C, T]) to every core; each core computes the
LayerNorm over all T locally (redundantly), so the expensive h1
AllGather of the previous version is eliminated. The only collective is
a small bf16 AllToAll of the per-head attention outputs.

Everything is bf16 (weights host-cast) except PSUM accumulation and LN
statistics. Attention scores use PE row-tiling: head0 occupies
partitions 0:64 and head1 64:128 of q/k, so one 512-col moving pass
computes both heads' score blocks concurrently. The two heads' PV
accumulators share one PSUM bank ([128, 130] = 2 x (64 o + 1 l)) using
per-element has_written semantics (single start=True clears the bank).
"""
import numpy as np
import ml_dtypes

import concourse.bass as bass
import concourse.bacc as bacc
import concourse.tile as tile
import concourse.mybir as mybir
from concourse import bass_utils

F32 = mybir.dt.float32
BF16 = mybir.dt.bfloat16
AF = mybir.ActivationFunctionType
OP = mybir.AluOpType

NCORES = 8
C = 1024
T = 4096
TC = T // NCORES          # 512 own tokens
CK = C // 128             # 8 C chunks
FC = 4096                 # MLP hidden
EPS = 1e-5

_CACHE = {}


def _layer_norm_own(nc, tc, sb, x_t, w_ap, out_t, eps_t, ones_t, tag):
    """LN over C for the own-column tile x_t [128, CK*512] bf16 (chunk k at
    cols 512k). out_t bf16 same layout. Stats in fp32."""
    with tc.tile_pool(name=f"ln{tag}", bufs=1) as lnp, \
         tc.tile_pool(name=f"lnps{tag}", bufs=1, space="PSUM") as lps:
        mean_ps = lps.tile([128, 512], F32, name=f"mean{tag}")
        sq_ps = lps.tile([128, 512], F32, name=f"sqs{tag}")
        for k in range(CK):
            xs = x_t[:, 512 * k:512 * (k + 1)]
            nc.tensor.matmul(mean_ps[:], ones_t[:], xs,
                             start=(k == 0), stop=(k == CK - 1))
            sq = lnp.tile([128, 512], BF16, name=f"sq{tag}", bufs=4)
            nc.vector.tensor_mul(sq[:], xs, xs)
            nc.tensor.matmul(sq_ps[:], ones_t[:], sq[:],
                             start=(k == 0), stop=(k == CK - 1))
        mu = lnp.tile([128, 512], F32, name=f"mu{tag}")
        nc.vector.tensor_scalar_mul(mu[:], mean_ps[:], 1.0 / C)
        musq = lnp.tile([128, 512], F32, name=f"musq{tag}")
        nc.vector.tensor_mul(musq[:], mu[:], mu[:])
        var = lnp.tile([128, 512], F32, name=f"var{tag}")
        nc.vector.scalar_tensor_tensor(var[:], sq_ps[:], 1.0 / C, musq[:],
                                       OP.mult, OP.subtract)
        lnv = lnp.tile([128, 512], F32, name=f"lnv{tag}")
        nc.scalar.activation(lnv[:], var[:], AF.Ln, bias=eps_t[:])
        rstd = lnp.tile([128, 512], F32, name=f"rstd{tag}")
        nc.scalar.activation(rstd[:], lnv[:], AF.Exp, scale=-0.5)
        for k in range(CK):
            d = lnp.tile([128, 512], F32, name=f"d{tag}", bufs=4)
            nc.vector.tensor_sub(d[:], x_t[:, 512 * k:512 * (k + 1)], mu[:])
            nc.vector.scalar_tensor_tensor(
                out_t[:, 512 * k:512 * (k + 1)], d[:], w_ap[:, k:k + 1],
                rstd[:], OP.mult, OP.mult)


def _build():
    nc = bacc.Bacc("TRN2", target_bir_lowering=False, debug=False,
                   enable_asserts=False, num_devices=NCORES)

    xT = nc.dram_tensor("xT", [C, T], BF16, kind="ExternalInput").ap()
    xoT = nc.dram_tensor("xoT", [C, TC], BF16, kind="ExternalInput").ap()
    wqkv = nc.dram_tensor("wqkv", [C, 384], BF16, kind="ExternalInput").ap()
    wqs = nc.dram_tensor("wqs", [1, 384], BF16, kind="ExternalInput").ap()
    wproj = nc.dram_tensor("wproj", [C, C], BF16, kind="ExternalInput").ap()
    wfc = nc.dram_tensor("wfc", [C, FC], BF16, kind="ExternalInput").ap()
    wmlp = nc.dram_tensor("wmlp", [FC, C], BF16, kind="ExternalInput").ap()
    ln1w = nc.dram_tensor("ln1w", [128, CK], F32, kind="ExternalInput").ap()
    ln2w = nc.dram_tensor("ln2w", [128, CK], F32, kind="ExternalInput").ap()
    masks = nc.dram_tensor("masks", [128, 4 * 1024], BF16,
                           kind="ExternalInput").ap()
    ident = nc.dram_tensor("ident", [128, 128], BF16, kind="ExternalInput").ap()
    onesw = nc.dram_tensor("onesw", [128, 128], BF16, kind="ExternalInput").ap()
    outT = nc.dram_tensor("outT", [C, TC], BF16, kind="ExternalOutput").ap()

    rg = [list(range(NCORES))]

    with tile.TileContext(nc) as tc:
        with tc.tile_pool(name="dram", bufs=1, space="DRAM") as dramp:
            ob = dramp.tile([C, TC], BF16, name="ob")
            oax = dramp.tile([C, TC], BF16, name="oax")

            with tc.tile_pool(name="glob", bufs=1) as gp:
                ident_t = gp.tile([128, 128], BF16, name="ident_t")
                nc.sync.dma_start(ident_t[:], ident[:])
                ones_t = gp.tile([128, 128], BF16, name="ones_t")
                nc.sync.dma_start(ones_t[:], onesw[:])
                ln1w_t = gp.tile([128, CK], F32, name="ln1w_t")
                nc.sync.dma_start(ln1w_t[:], ln1w[:])
                ln2w_t = gp.tile([128, CK], F32, name="ln2w_t")
                nc.sync.dma_start(ln2w_t[:], ln2w[:])
                eps_t = gp.tile([128, 1], F32, name="eps_t")
                nc.vector.memset(eps_t[:], EPS)
                masks_t = gp.tile([128, 4 * 1024], BF16, name="masks_t")
                x_own = gp.tile([128, CK * TC], BF16, name="x_own")
                oT = gp.tile([128, T], BF16, name="oT")

                # ---------------- attention scope ----------------
                with tc.tile_pool(name="attn", bufs=1) as ap:
                    qp = ap.tile([128, T], BF16, name="qp")
                    kT = ap.tile([128, T], BF16, name="kT")
                    vT = ap.tile([128, T], BF16, name="vT")

                    # LN1 over FULL T + QKV for the 2 local heads. The LN
                    # apply is FOLDED into the QKV matmul: qkv = w'^T x -
                    # s (x) mu (rank-1 K=1 correction, w' = w*gamma and
                    # s = colsum(w') precomputed on host), then the
                    # psum->SBUF copy multiplies by rstd per column. So
                    # the matmuls consume RAW x and no h1 is materialized.
                    with tc.tile_pool(name="p1", bufs=1) as p1:
                        x_t = p1.tile([128, CK * T], BF16, name="x_t")
                        mu = p1.tile([128, T], BF16, name="mu")
                        rstd = p1.tile([128, T], BF16, name="rstd")
                        wqs_t = p1.tile([1, 384], BF16, name="wqs_t")
                        nc.sync.dma_start(wqs_t[:], wqs[:])
                        # x chunks first (LN1 stats are the critical path),
                        # split in halves for earlier first-arrival
                        for k in range(CK):
                            for hh in range(2):
                                nc.sync.dma_start(
                                    x_t[:, T * k + 2048 * hh:
                                        T * k + 2048 * (hh + 1)],
                                    xT[128 * k:128 * (k + 1),
                                       2048 * hh:2048 * (hh + 1)])
                        wq_t = []
                        for k in range(CK):
                            w = p1.tile([128, 384], BF16, name=f"wq{k}")
                            nc.sync.dma_start(w[:],
                                              wqkv[128 * k:128 * (k + 1), :])
                            wq_t.append(w)
                        nc.sync.dma_start(masks_t[:], masks[:])
                        for k in range(CK):
                            nc.sync.dma_start(x_own[:, 512 * k:512 * (k + 1)],
                                              xoT[128 * k:128 * (k + 1), :])
                        # stats in 2 halves of T (PSUM: 4+4 banks per half);
                        # squares computed chunk-wide (1 DVE op per chunk)
                        with tc.tile_pool(name="lnps1", bufs=1,
                                          space="PSUM") as lps:
                            for half in range(2):
                                mps = [lps.tile([128, 512], F32, name=f"m{tb}",
                                                bufs=1) for tb in range(4)]
                                sps = [lps.tile([128, 512], F32, name=f"s{tb}",
                                                bufs=1) for tb in range(4)]
                                for k in range(CK):
                                    xh = x_t[:, T * k + 2048 * half:
                                             T * k + 2048 * (half + 1)]
                                    sq = p1.tile([128, 2048], BF16,
                                                 name="sq1", bufs=2)
                                    nc.vector.tensor_mul(sq[:], xh, xh)
                                    for tb in range(4):
                                        col = T * k + 2048 * half + 512 * tb
                                        nc.tensor.matmul(
                                            mps[tb][:], ones_t[:],
                                            x_t[:, col:col + 512],
                                            start=(k == 0), stop=(k == CK - 1))
                                        nc.tensor.matmul(
                                            sps[tb][:], ones_t[:],
                                            sq[:, 512 * tb:512 * (tb + 1)],
                                            start=(k == 0), stop=(k == CK - 1))
                                for tb in range(4):
                                    tcol = 2048 * half + 512 * tb
                                    mus = mu[:, tcol:tcol + 512]
                                    nc.vector.tensor_scalar_mul(
                                        mus, mps[tb][:], 1.0 / C)
                                    musq = p1.tile([128, 512], F32,
                                                   name="musq1", bufs=2)
                                    nc.vector.tensor_mul(musq[:], mus, mus)
                                    var = p1.tile([128, 512], F32,
                                                  name="var1", bufs=2)
                                    nc.vector.scalar_tensor_tensor(
                                        var[:], sps[tb][:], 1.0 / C, musq[:],
                                        OP.mult, OP.subtract)
                                    lnv = p1.tile([128, 512], F32,
                                                  name="lnv1", bufs=2)
                                    nc.scalar.activation(lnv[:], var[:],
                                                         AF.Ln, bias=eps_t[:])
                                    nc.scalar.activation(
                                        rstd[:, tcol:tcol + 512], lnv[:],
                                        AF.Exp, scale=-0.5)
                        # qkv on RAW x with rank-1 LN correction; the
                        # psum->SBUF copy applies the rstd column scaling.
                        dsts = [qp, kT, vT]
                        with tc.tile_pool(name="qkps", bufs=1,
                                          space="PSUM") as qps:
                            for j in range(8):
                                for m in range(3):
                                    pm = qps.tile([128, 512], F32,
                                                  name=f"pm{m}", bufs=2)
                                    for k in range(CK):
                                        nc.tensor.matmul(
                                            pm[:],
                                            wq_t[k][:, 128 * m:128 * (m + 1)],
                                            x_t[:, T * k + 512 * j:
                                                T * k + 512 * (j + 1)],
                                            start=(k == 0), stop=False)
                                    nc.tensor.matmul(
                                        pm[:],
                                        wqs_t[0:1, 128 * m:128 * (m + 1)],
                                        mu[0:1, 512 * j:512 * (j + 1)],
                                        start=False, stop=True)
                                    nc.vector.tensor_mul(
                                        dsts[m][:, 512 * j:512 * (j + 1)],
                                        pm[:],
                                        rstd[:, 512 * j:512 * (j + 1)])

                    # v transposed per head: ve[h] [128, 32*65] bf16
                    # (block kb at cols 65*kb; col 64 of each block = ones)
                    ve = [ap.tile([128, 32 * 65], BF16, name=f"ve{h}")
                          for h in range(2)]
                    for h in range(2):
                        nc.vector.memset(ve[h][:], 1.0)
                    with tc.tile_pool(name="veps", bufs=1, space="PSUM") as vps:
                        for t in range(T // 128):
                            tp = vps.tile([128, 128], BF16, name="vtp", bufs=4)
                            nc.tensor.transpose(tp[:],
                                                vT[:, 128 * t:128 * (t + 1)],
                                                ident_t[:])
                            for h in range(2):
                                nc.vector.tensor_copy(
                                    ve[h][:, 65 * t:65 * t + 64],
                                    tp[:, 64 * h:64 * (h + 1)])

                    # flash attention (no max subtraction), both heads/pass
                    o2_tiles = []
                    with tc.tile_pool(name="atw", bufs=1) as aw:
                      with tc.tile_pool(name="atps", bufs=1,
                                        space="PSUM") as aps:
                        for qi in range(8):
                            nkb = 4 * (qi + 1)
                            ops = [aps.tile([128, 130], F32, name=f"ops{q}",
                                            bufs=1) for q in range(4)]
                            for kb in range(nkb):
                                st = aps.tile([128, 1024], F32, name="st",
                                              bufs=2)
                                for h in range(2):
                                    nc.tensor.matmul(
                                        st[:, 512 * h:512 * (h + 1)],
                                        kT[64 * h:64 * (h + 1),
                                           128 * kb:128 * (kb + 1)],
                                        qp[64 * h:64 * (h + 1),
                                           512 * qi:512 * (qi + 1)],
                                        start=True, stop=True)
                                est = aw.tile([128, 1024], BF16, name="est",
                                              bufs=4)
                                nc.scalar.activation(est[:], st[:], AF.Exp,
                                                     scale=0.125)
                                if kb >= 4 * qi:
                                    jm = kb - 4 * qi
                                    nc.vector.tensor_mul(
                                        est[:], est[:],
                                        masks_t[:, 1024 * jm:1024 * (jm + 1)])
                                last = (kb == nkb - 1)
                                for q in range(4):
                                    nc.tensor.matmul(
                                        ops[q][:, 0:65],
                                        est[:, 128 * q:128 * (q + 1)],
                                        ve[0][:, 65 * kb:65 * kb + 65],
                                        start=(kb == 0), stop=last,
                                        skip_group_check=True)
                                    nc.tensor.matmul(
                                        ops[q][:, 65:130],
                                        est[:, 512 + 128 * q:512 + 128 * (q + 1)],
                                        ve[1][:, 65 * kb:65 * kb + 65],
                                        start=False, stop=last,
                                        skip_group_check=True)
                            # normalize: o2[q] = [o_h0 * 1/l_h0 | o_h1 * 1/l_h1]
                            l_sb = aw.tile([128, 8], F32, name="l_sb", bufs=2)
                            for q in range(4):
                                for h in range(2):
                                    nc.vector.tensor_copy(
                                        l_sb[:, 2 * q + h:2 * q + h + 1],
                                        ops[q][:, 65 * h + 64:65 * h + 65])
                            rl = aw.tile([128, 8], F32, name="rl", bufs=2)
                            nc.vector.reciprocal(rl[:], l_sb[:])
                            for q in range(4):
                                o2 = aw.tile([128, 128], BF16,
                                             name=f"o2_{qi}_{q}")
                                for h in range(2):
                                    nc.vector.tensor_scalar_mul(
                                        o2[:, 64 * h:64 * (h + 1)],
                                        ops[q][:, 65 * h:65 * h + 64],
                                        rl[:, 2 * q + h:2 * q + h + 1])
                                o2_tiles.append(o2)
                      # transpose all o2 -> oT [d2, q] (atps closed: banks free)
                      with tc.tile_pool(name="trps", bufs=1,
                                        space="PSUM") as tq:
                          for i, o2 in enumerate(o2_tiles):
                              tp2 = tq.tile([128, 128], BF16, name="tp2",
                                            bufs=4)
                              nc.tensor.transpose(tp2[:], o2[:], ident_t[:])
                              nc.vector.tensor_copy(
                                  oT[:, 128 * i:128 * (i + 1)], tp2[:])

                # tail scope: proj + MLP. Weight prefetch DMAs are issued
                # BEFORE the collective so they overlap the a2a latency.
                with tc.tile_pool(name="tail", bufs=1) as tp:
                    # exchange head outputs: AllToAll (bf16). The proj/fc
                    # weight prefetch DMAs are issued right after the
                    # trigger so they overlap the collective's latency.
                    for j in range(NCORES):
                        nc.sync.dma_start(ob[128 * j:128 * (j + 1), :],
                                          oT[:, 512 * j:512 * (j + 1)])
                    nc.gpsimd.collective_compute(
                        "AllToAll", OP.bypass, replica_groups=rg,
                        ins=[ob.opt()], outs=[oax.opt()])

                    x2_t = tp.tile([128, CK * TC], BF16, name="x2_t")
                    out_sb = tp.tile([128, CK * TC], BF16, name="out_sb")
                    wp_t = []
                    for k in range(CK):
                        w = tp.tile([128, C], BF16, name=f"wp{k}")
                        nc.sync.dma_start(w[:],
                                          wproj[128 * k:128 * (k + 1), :])
                        wp_t.append(w)
                    wfc_r = wfc.rearrange("(k p) h -> p k h", p=128)
                    wg_t = []
                    for m in range(FC // 128):
                        w = tp.tile([128, CK, 128], BF16, name=f"wg{m}")
                        nc.sync.dma_start(
                            w[:], wfc_r[:, :, 128 * m:128 * (m + 1)])
                        wg_t.append(w)

                    # proj: x2 = x_own + wproj.T @ aout (own columns)
                    with tc.tile_pool(name="prs", bufs=1) as prs, \
                         tc.tile_pool(name="prps", bufs=1, space="PSUM") as pps:
                        x2ps = [pps.tile([128, 512], F32, name=f"x2p{m}")
                                for m in range(CK)]
                        for k in range(CK):
                            at = prs.tile([128, 512], BF16, name="at", bufs=3)
                            nc.sync.dma_start(at[:],
                                              oax[128 * k:128 * (k + 1), :])
                            for m in range(CK):
                                nc.tensor.matmul(
                                    x2ps[m][:],
                                    wp_t[k][:, 128 * m:128 * (m + 1)],
                                    at[:], start=(k == 0), stop=(k == CK - 1))
                        for m in range(CK):
                            nc.vector.tensor_add(
                                x2_t[:, 512 * m:512 * (m + 1)], x2ps[m][:],
                                x_own[:, 512 * m:512 * (m + 1)])

                    # ---------------- MLP ----------------
                    with tc.tile_pool(name="mlp", bufs=1) as mp:
                        h2 = mp.tile([128, CK * TC], BF16, name="h2")
                        _layer_norm_own(nc, tc, mp, x2_t, ln2w_t, h2, eps_t,
                                        ones_t, "2")
                        gel = []
                        with tc.tile_pool(name="fcps", bufs=1,
                                          space="PSUM") as fps:
                            for m in range(FC // 128):
                                pf = fps.tile([128, 512], F32, name="fcp",
                                              bufs=2)
                                for k in range(CK):
                                    nc.tensor.matmul(
                                        pf[:], wg_t[m][:, k, :],
                                        h2[:, 512 * k:512 * (k + 1)],
                                        start=(k == 0), stop=(k == CK - 1))
                                g = mp.tile([128, 512], BF16, name=f"gel{m}")
                                nc.scalar.activation(g[:], pf[:], AF.Gelu)
                                gel.append(g)
                        # second matmul in two half-passes (PSUM budget)
                        with tc.tile_pool(name="m2s", bufs=1) as m2s, \
                             tc.tile_pool(name="m2ps", bufs=1,
                                          space="PSUM") as m2ps:
                            for half in range(2):
                                x3ps = [m2ps.tile([128, 512], F32,
                                                  name=f"x3p{i}", bufs=1)
                                        for i in range(4)]
                                for h in range(FC // 128):
                                    wm = m2s.tile([128, 512], BF16, name="wm",
                                                  bufs=8)
                                    nc.sync.dma_start(
                                        wm[:],
                                        wmlp[128 * h:128 * (h + 1),
                                             512 * half:512 * (half + 1)])
                                    for i in range(4):
                                        nc.tensor.matmul(
                                            x3ps[i][:],
                                            wm[:, 128 * i:128 * (i + 1)],
                                            gel[h][:], start=(h == 0),
                                            stop=(h == FC // 128 - 1))
                                for i in range(4):
                                    m = 4 * half + i
                                    nc.vector.tensor_add(
                                        out_sb[:, 512 * m:512 * (m + 1)],
                                        x3ps[i][:],
                                        x2_t[:, 512 * m:512 * (m + 1)])
                                    nc.sync.dma_start(
                                        outT[128 * m:128 * (m + 1), :],
                                        out_sb[:, 512 * m:512 * (m + 1)])

    nc.compile()
    return nc


def _host_inputs(x, w_qkv, w_attn_proj, w_fc, w_mlp_proj, ln1_w, ln2_w):
    """Build the 8 per-core input maps (bf16 weights/activations)."""
    bf16 = ml_dtypes.bfloat16
    x2 = np.ascontiguousarray(np.asarray(x, np.float32).reshape(T, C))
    xT_full = np.ascontiguousarray(x2.T).astype(bf16)
    w_qkv = np.asarray(w_qkv, np.float32)
    masks = np.zeros((128, 4 * 1024), np.float32)
    kk = np.arange(128)[:, None]
    qq = np.arange(512)[None, :]
    for jm in range(4):
        m = (qq >= kk + 128 * jm)
        masks[:, 1024 * jm:1024 * jm + 512] = m
        masks[:, 1024 * jm + 512:1024 * (jm + 1)] = m
    masks = masks.astype(bf16)
    ident = np.eye(128, dtype=np.float32).astype(bf16)
    onesw = np.ones((128, 128), np.float32).astype(bf16)
    ln1 = np.ascontiguousarray(np.asarray(ln1_w, np.float32).reshape(CK, 128).T)
    ln2 = np.ascontiguousarray(np.asarray(ln2_w, np.float32).reshape(CK, 128).T)
    common = {
        "xT": xT_full,
        "wproj": np.ascontiguousarray(
            np.asarray(w_attn_proj, np.float32)).astype(bf16),
        "wfc": np.ascontiguousarray(np.asarray(w_fc, np.float32)).astype(bf16),
        "wmlp": np.ascontiguousarray(
            np.asarray(w_mlp_proj, np.float32)).astype(bf16),
        "ln1w": ln1, "ln2w": ln2, "masks": masks, "ident": ident,
        "onesw": onesw,
    }
    ln1f = np.asarray(ln1_w, np.float32).reshape(C, 1)
    in_maps = []
    for c in range(NCORES):
        xoT = np.ascontiguousarray(x2[TC * c:TC * (c + 1), :].T).astype(bf16)
        wqf = np.ascontiguousarray(np.concatenate(
            [w_qkv[:, C * s + 128 * c:C * s + 128 * (c + 1)] for s in range(3)],
            axis=1)) * ln1f
        wq = wqf.astype(bf16)
        wqsum = np.ascontiguousarray(
            -wq.astype(np.float32).sum(axis=0, keepdims=True)).astype(bf16)
        in_maps.append({"xoT": xoT, "wqkv": wq, "wqs": wqsum, **common})
    return in_maps


def _run(in_maps, **kw):
    if "nc" not in _CACHE:
        _CACHE["nc"] = _build()
    return bass_utils.run_bass_kernel_spmd(
        _CACHE["nc"], in_maps, core_ids=list(range(NCORES)), **kw)


def kernel(x, w_qkv, w_attn_proj, w_fc, w_mlp_proj, ln1_w, ln2_w):
    in_maps = _host_inputs(x, w_qkv, w_attn_proj, w_fc, w_mlp_proj,
                           ln1_w, ln2_w)
    res = _run(in_maps)
    out = np.empty((1, T, C), np.float32)
    for c in range(NCORES):
        out[0, TC * c:TC * (c + 1), :] = \
            res.results[c]["outT"].astype(np.float32).T
    return out
